# revision 1
# baseline (speedup 1.0000x reference)
"""Bass/Trainium2 kernel for a 2-layer GAT (nn_GAT_48919677501958).

Contract: kernel(**inputs) takes the FULL unsharded numpy inputs (keyed as in
setup_inputs()) and returns the FULL [10000, 40] float32 output.

Strategy (8 NeuronCores, SPMD single program):
  - Host: append self-loops, sort nodes by in-degree, bucket into 128-node
    tiles (uniform degree per tile), snake-assign 10 tiles to each core.
    All graph structure is baked into per-core *data* inputs (int16 gather
    index lists + float masks); the program is identical on every core.
  - Device per core:
      Phase A: full H = X@W1 (+ attention-half columns) replicated on every
               core, written to DRAM as gather table `Haug` [10240, 576]:
               row = [h(512) | alpha_src(8) | pad]. fp32r matmuls.
      Phase B: per own dst-tile: dma_gather of neighbor rows (dst-major:
               edge slots on the free dim, dst nodes on partitions), masked
               segment softmax (LeakyReLU+Exp on ACT, row reductions on DVE),
               alpha-weighted message sum (in-place DVE mul + tree fold),
               ELU -> y tile.
      Phase C: PE-transpose y, layer-2 matmul -> Haug2_own [1280, 64]
               rows = [h2(40) | alpha_src2(1) | alpha_dst2(1) | pad].
      AllGather Haug2 across the 8 cores (the only collective).
      Phase D: layer-2 edge phase (same index lists, 256B rows) -> out rows.
  - Host: concat per-core outputs, inverse-permute rows.
"""

import math
from dataclasses import dataclass, field

import numpy as np

import concourse.bass as bass
import concourse.mybir as mybir
import concourse.tile as tile
from concourse.bass_utils import run_bass_kernel_spmd
from concourse.masks import make_identity

F32 = mybir.dt.float32
F32R = mybir.dt.float32r
I16 = mybir.dt.int16

NEG_SLOPE = 0.2
P = 128  # partitions


@dataclass
class Cfg:
    n_nodes: int  # real node count (10000)
    n_cores: int  # 8
    tpc: int  # tiles per core (10)
    d_in: int  # 256
    hid: int  # 64
    heads: int  # 8
    d_out: int  # 40
    ch: int  # max gather chunk slots (L1)
    s_prog: list[int] = field(default_factory=list)  # slots per tile rank
    mm_mode: str = "f32r"  # f32 | f32r | bf16
    tab_dt: str = "f32"  # gather-table dtype: f32 | bf16
    phases: str = "ABCD"  # phase prefix to emit (model/debug)
    collective: bool = True  # False: replace AllGather with local copy (model)
    shard_a: bool = True  # shard phase A across cores + AllGather the table
    hg_bufs: int = 2
    ilv: bool = True  # head-interleaved table layout (col = c*heads + h)
    lrelu_act: bool = False  # ACT Lrelu LUT: table reloads cost more than DVE decomp
    b_level: int = 9  # model bisect: 1=gather 2=+softmax 3=+mul 4=+tree/agg
    d_level: int = 9  # model bisect: 1=gather 2=+softmax 3=+mul 4=+tree/out

    @property
    def npc(self):  # padded nodes per core
        return self.tpc * P

    @property
    def npad(self):
        return self.n_cores * self.npc

    @property
    def d_hid(self):  # concat hidden width (512)
        return self.hid * self.heads

    @property
    def rw1(self):  # Haug row width in elements (row bytes must be %256)
        q = 64 if self.tab_dt == "f32" else 128
        return ((self.d_hid + self.heads) + q - 1) // q * q

    @property
    def rw2(self):  # Haug2 row width in f32
        return 64

    @property
    def sum_s(self):
        return sum(self.s_prog)

    def chunks(self, t):
        s = self.s_prog[t]
        out = []
        while s > 0:
            c = min(s, self.ch)
            out.append(c)
            s -= c
        return out


def _wrap_idx(flat: np.ndarray) -> np.ndarray:
    """dma_gather index layout: position i lives at [i % 16, i // 16],
    replicated across the 8 GpSimd-core stripes of 16 partitions each."""
    assert flat.size % 16 == 0
    w = np.ascontiguousarray(flat.reshape(-1, 16).T).astype(np.int16)
    return np.tile(w, (8, 1))


def preprocess(cfg: Cfg, x, edge_index, W1, att_src1, att_dst1, b1, W2,
               att_src2, att_dst2, b2):
    """Host-side graph/layout preprocessing. Returns (in_maps, node_of_slot)."""
    N = cfg.n_nodes
    src = np.concatenate([np.asarray(edge_index[0], np.int64), np.arange(N)])
    dst = np.concatenate([np.asarray(edge_index[1], np.int64), np.arange(N)])
    deg = np.bincount(dst, minlength=N)

    # CSR by dst
    order_e = np.argsort(dst, kind="stable")
    sorted_src = src[order_e]
    starts = np.zeros(N + 1, np.int64)
    np.cumsum(deg, out=starts[1:])

    # degree-sorted node order, padded with -1 dummies to npad slots
    node_order = np.argsort(-deg, kind="stable")
    padded = np.full(cfg.npad, -1, np.int64)
    padded[:N] = node_order
    tiles = padded.reshape(-1, P)  # [n_tiles_total, 128]
    n_tiles = tiles.shape[0]
    assert n_tiles == cfg.n_cores * cfg.tpc
    tile_deg = np.where(tiles >= 0, deg[np.maximum(tiles, 0)], 0)
    tile_s = np.maximum(tile_deg.max(axis=1), 1)  # >=1 slot per tile

    # snake-assign tiles to cores, then per-core sort tiles by S desc
    core_tiles = [[] for _ in range(cfg.n_cores)]
    for r in range(cfg.tpc):
        row = list(range(r * cfg.n_cores, (r + 1) * cfg.n_cores))
        if r % 2:
            row = row[::-1]
        for c in range(cfg.n_cores):
            core_tiles[c].append(row[c])
    for c in range(cfg.n_cores):
        core_tiles[c].sort(key=lambda i: -tile_s[i])

    cfg.s_prog = [
        int(max(tile_s[core_tiles[c][t]] for c in range(cfg.n_cores)))
        for t in range(cfg.tpc)
    ]

    if cfg.tab_dt == "bf16":
        cfg.ch = max(cfg.s_prog)  # whole tile in one gather

    # slot -> node map and node -> Haug row map
    node_of_slot = np.full((cfg.n_cores, cfg.npc), -1, np.int64)
    for c in range(cfg.n_cores):
        for t in range(cfg.tpc):
            node_of_slot[c, t * P:(t + 1) * P] = tiles[core_tiles[c][t]]
    row_of_node = np.full(N, -1, np.int64)
    flat_slots = node_of_slot.reshape(-1)
    real = flat_slots >= 0
    row_of_node[flat_slots[real]] = np.nonzero(real)[0]
    assert (row_of_node >= 0).all()

    # permuted, padded, transposed x
    xT = np.zeros((cfg.d_in, cfg.npad), np.float32)
    xT[:, np.nonzero(real)[0]] = np.asarray(x, np.float32).T[:, flat_slots[real]]

    # packed weights (host weight-folding only)
    W1 = np.asarray(W1, np.float32)
    ablk_s = np.zeros((cfg.d_hid, cfg.heads), np.float32)
    ablk_d = np.zeros((cfg.d_hid, cfg.heads), np.float32)
    a_s1 = np.asarray(att_src1, np.float32)
    a_d1 = np.asarray(att_dst1, np.float32)
    for h in range(cfg.heads):
        ablk_s[h * cfg.hid:(h + 1) * cfg.hid, h] = a_s1[h]
        ablk_d[h * cfg.hid:(h + 1) * cfg.hid, h] = a_d1[h]
    Wa1 = np.concatenate([W1 @ ablk_s, W1 @ ablk_d], axis=1)  # [d_in, 2*heads]
    W2 = np.asarray(W2, np.float32)
    w2s = W2 @ np.asarray(att_src2, np.float32)[0]
    w2d = W2 @ np.asarray(att_dst2, np.float32)[0]
    W2a = np.concatenate([W2, w2s[:, None], w2d[:, None]], axis=1)  # [512, 42]
    b1r = np.tile(np.asarray(b1, np.float32)[None, :], (P, 1))
    b2r = np.tile(np.asarray(b2, np.float32)[None, :], (P, 1))
    if cfg.ilv and cfg.tab_dt == "bf16":
        # interleaved hidden layout: new col j=(c,h) maps to old col h*hid+c
        j = np.arange(cfg.d_hid)
        old = (j % cfg.heads) * cfg.hid + j // cfg.heads
        b1r = np.ascontiguousarray(b1r[:, old])
        W2a = np.ascontiguousarray(W2a[old, :])
    else:
        cfg.ilv = False

    # per-core gather indices + masks
    in_maps = []
    for c in range(cfg.n_cores):
        gi1_parts, gi2_parts, gm_parts = [], [], []
        for t in range(cfg.tpc):
            s_t = cfg.s_prog[t]
            nodes = node_of_slot[c, t * P:(t + 1) * P]
            srcs = np.zeros((P, s_t), np.int64)
            mask = np.zeros((P, s_t), np.float32)
            for d in range(P):
                n = nodes[d]
                if n >= 0:
                    k = deg[n]
                    srcs[d, :k] = row_of_node[sorted_src[starts[n]:starts[n] + k]]
                    mask[d, :k] = 1.0
                else:
                    mask[d, 0] = 1.0  # keep denom > 0 on dummy rows
            off = 0
            for s_c in cfg.chunks(t):
                gi1_parts.append(srcs[:, off:off + s_c].T.reshape(-1))
                off += s_c
            gi2_parts.append(srcs.T.reshape(-1))
            gm_parts.append(mask)
        gi1 = _wrap_idx(np.concatenate(gi1_parts))
        gi2 = _wrap_idx(np.concatenate(gi2_parts))
        gm = np.concatenate(gm_parts, axis=1)
        in_maps.append({
            "xT": xT,
            "xTo": np.ascontiguousarray(xT[:, c * cfg.npc:(c + 1) * cfg.npc]),
            "W1": W1, "Wa1": Wa1, "b1r": b1r, "W2a": W2a, "b2r": b2r,
            "gi1": gi1, "gi2": gi2, "gm": gm,
        })
    return in_maps, node_of_slot


def build_program(cfg: Cfg) -> bass.Bass:
    import concourse.bacc as bacc
    nc = bacc.Bacc("TRN2", target_bir_lowering=False, num_devices=cfg.n_cores)
    DH, HD = cfg.d_hid, cfg.heads
    KT = cfg.d_in // P  # k-tiles for layer-1 matmul
    K2 = DH // P        # k-tiles for layer-2 matmul
    n_tiles_all = cfg.npad // P
    NIDX = P * cfg.sum_s

    MMDT = {"f32": F32, "f32r": F32R, "bf16": mybir.dt.bfloat16}[cfg.mm_mode]
    TDT = {"f32": F32, "bf16": mybir.dt.bfloat16}[cfg.tab_dt]
    def mm_load(out_ap, in_ap):
        # DMA that casts f32 DRAM -> matmul dtype in SBUF (SWDGE casts)
        if cfg.mm_mode == "f32":
            nc.sync.dma_start(out=out_ap, in_=in_ap)
        else:
            nc.gpsimd.dma_start(out=out_ap, in_=in_ap)

    # ---- DRAM ----
    xT = (None if cfg.shard_a else
          nc.dram_tensor("xT", [cfg.d_in, cfg.npad], F32, kind="ExternalInput"))
    xTo = nc.dram_tensor("xTo", [cfg.d_in, cfg.npc], F32, kind="ExternalInput")
    W1 = nc.dram_tensor("W1", [cfg.d_in, DH], F32, kind="ExternalInput")
    Wa1 = nc.dram_tensor("Wa1", [cfg.d_in, 2 * HD], F32, kind="ExternalInput")
    b1r = nc.dram_tensor("b1r", [P, DH], F32, kind="ExternalInput")
    W2a = nc.dram_tensor("W2a", [DH, cfg.d_out + 2], F32, kind="ExternalInput")
    b2r = nc.dram_tensor("b2r", [P, cfg.d_out], F32, kind="ExternalInput")
    gi1 = nc.dram_tensor("gi1", [P, NIDX // 16], I16, kind="ExternalInput")
    gi2 = nc.dram_tensor("gi2", [P, NIDX // 16], I16, kind="ExternalInput")
    gm = nc.dram_tensor("gm", [P, cfg.sum_s], F32, kind="ExternalInput")
    out = nc.dram_tensor("out", [cfg.npc, cfg.d_out], F32, kind="ExternalOutput")

    haug = nc.dram_tensor("haug", [cfg.npad, cfg.rw1], TDT,
                          addr_space="Shared" if (cfg.shard_a and cfg.collective)
                          else "Local")
    haug_own = (nc.dram_tensor("haug_own", [cfg.npc, cfg.rw1], TDT)
                if cfg.shard_a else None)
    h2own = nc.dram_tensor("h2own", [cfg.npc, cfg.rw2], F32)
    h2all = nc.dram_tensor("h2all", [cfg.npad, cfg.rw2], F32, addr_space="Shared")

    from contextlib import ExitStack
    with tile.TileContext(nc) as tc, ExitStack() as st:
        cst = st.enter_context(tc.tile_pool(name="cst", bufs=1))
        lhs_p = st.enter_context(tc.tile_pool(name="lhs", bufs=4))
        hsb_p = st.enter_context(tc.tile_pool(name="hsb", bufs=4))
        psH_p = st.enter_context(tc.tile_pool(name="psH", bufs=2, space="PSUM"))
        psA_p = st.enter_context(tc.tile_pool(name="psA", bufs=2, space="PSUM"))
        hg_p = st.enter_context(tc.tile_pool(name="hg", bufs=cfg.hg_bufs))
        hg2_p = st.enter_context(tc.tile_pool(name="hg2p", bufs=2))
        sm_p = st.enter_context(tc.tile_pool(name="sm", bufs=6))
        big_p = st.enter_context(tc.tile_pool(name="big", bufs=3))
        out_p = st.enter_context(tc.tile_pool(name="outp", bufs=4))

        # ---- constants to SBUF ----
        w1sb = cst.tile([P, KT, DH], MMDT)
        wa1sb = cst.tile([P, KT, 2 * HD], MMDT)
        w2sb = cst.tile([P, K2, cfg.d_out + 2], MMDT)
        b1sb = cst.tile([P, DH], F32)
        b2sb = cst.tile([P, cfg.d_out], F32)
        gmsb = cst.tile([P, cfg.sum_s], F32)
        gi1sb = cst.tile([P, NIDX // 16], I16)
        gi2sb = cst.tile([P, NIDX // 16], I16)
        ident = cst.tile([P, P], F32)
        ad_sb = cst.tile([P, cfg.tpc * HD], F32)
        ad2_sb = cst.tile([P, cfg.tpc], F32)
        for k in range(KT):
            mm_load(w1sb[:, k, :], W1[k * P:(k + 1) * P, :])
            mm_load(wa1sb[:, k, :], Wa1[k * P:(k + 1) * P, :])
        for k in range(K2):
            mm_load(w2sb[:, k, :], W2a[k * P:(k + 1) * P, :])
        nc.sync.dma_start(out=b1sb[:], in_=b1r[:])
        nc.sync.dma_start(out=b2sb[:], in_=b2r[:])
        nc.sync.dma_start(out=gmsb[:], in_=gm[:])
        nc.sync.dma_start(out=gi1sb[:], in_=gi1[:])
        nc.sync.dma_start(out=gi2sb[:], in_=gi2[:])
        make_identity(nc, ident[:])

        # ---- Phase A ----
        def emit_a_tile(m, src_dram, dst_dram, keep_ad_t=None, lhs_sb=None):
            if lhs_sb is not None:
                lt = lhs_sb[:, :, m * P:(m + 1) * P]
            else:
                lt = lhs_p.tile([P, KT, P], MMDT, tag="lhs")
                for k in range(KT):
                    mm_load(lt[:, k, :],
                            src_dram[k * P:(k + 1) * P, m * P:(m + 1) * P])
            ph = psH_p.tile([P, DH], F32)
            pa = psA_p.tile([P, 2 * HD], F32)
            for k in range(KT):
                nc.tensor.matmul(ph[:], lt[:, k, :], w1sb[:, k, :],
                                 start=(k == 0), stop=(k == KT - 1))
            for k in range(KT):
                nc.tensor.matmul(pa[:], lt[:, k, :], wa1sb[:, k, :],
                                 start=(k == 0), stop=(k == KT - 1))
            hs = hsb_p.tile([P, cfg.rw1], TDT, tag="hsb")
            if cfg.ilv:
                nc.scalar.copy(
                    hs[:, :DH].rearrange("p (c h) -> p h c", h=HD),
                    ph[:].rearrange("p (h c) -> p h c", h=HD))
            else:
                nc.scalar.copy(hs[:, :DH], ph[:])
            nc.scalar.copy(hs[:, DH:DH + HD], pa[:, :HD])
            if cfg.rw1 > DH + HD:
                nc.vector.memset(hs[:, DH + HD:], 0.0)
            if keep_ad_t is not None:
                nc.scalar.copy(ad_sb[:, keep_ad_t * HD:(keep_ad_t + 1) * HD],
                               pa[:, HD:2 * HD])
            nc.sync.dma_start(out=dst_dram[m * P:(m + 1) * P, :], in_=hs[:])

        if cfg.shard_a:
            xosb = cst.tile([P, KT, cfg.npc], MMDT)
            for k in range(KT):
                mm_load(xosb[:, k, :], xTo[k * P:(k + 1) * P, :])
            for t in range(cfg.tpc):
                emit_a_tile(t, None, haug_own, keep_ad_t=t, lhs_sb=xosb)
            if cfg.collective:
                nc.gpsimd.collective_compute(
                    "AllGather", mybir.AluOpType.bypass,
                    ins=[haug_own[:]], outs=[haug[:]],
                    replica_groups=[list(range(cfg.n_cores))])
            else:
                nc.sync.dma_start(out=haug[0:cfg.npc, :], in_=haug_own[:])
        else:
            for m in range(n_tiles_all):
                emit_a_tile(m, xT, haug)
            # own alpha_dst rows
            for t in range(cfg.tpc):
                lt = lhs_p.tile([P, KT, P], MMDT, tag="lhs")
                for k in range(KT):
                    mm_load(lt[:, k, :], xTo[k * P:(k + 1) * P, t * P:(t + 1) * P])
                pa = psA_p.tile([P, 2 * HD], F32)
                for k in range(KT):
                    nc.tensor.matmul(pa[:, :HD], lt[:, k, :],
                                     wa1sb[:, k, HD:2 * HD],
                                     start=(k == 0), stop=(k == KT - 1))
                nc.scalar.copy(ad_sb[:, t * HD:(t + 1) * HD], pa[:, :HD])

        # ---- Phases B + C per own tile ----
        doB = "B" in cfg.phases
        doC = "C" in cfg.phases
        doD = "D" in cfg.phases
        gi_off = 0  # in index positions
        gm_off = 0
        for t in range(cfg.tpc if doB else 0):
            s_t = cfg.s_prog[t]
            chunks = cfg.chunks(t)
            if len(chunks) > 1:
                agg = big_p.tile([P, DH], F32, tag="agg")
            else:
                agg = None
            EXDT = mybir.dt.bfloat16 if cfg.ilv else F32
            ex_all = sm_p.tile([P, cfg.s_prog[0], HD], EXDT, tag="ex")
            hgs = []
            c_off = 0
            for ci, s_c in enumerate(chunks):
                nidx = P * s_c
                hg = hg_p.tile([P, cfg.ch, cfg.rw1], TDT, tag="hg")
                hgv = hg[:, :s_c, :]
                nc.gpsimd.dma_gather(
                    out_ap=hgv,
                    in_ap=haug[:, :],
                    idxs_ap=gi1sb[:, gi_off // 16:(gi_off + nidx) // 16],
                    num_idxs=nidx, num_idxs_reg=nidx, elem_size=cfg.rw1,
                    single_packet=False)
                gi_off += nidx
                hgs.append((hgv, s_c, c_off))
                if cfg.b_level >= 2:
                    # e = lrelu(alpha_src[src] + alpha_dst[dst]); ex = exp(e)*mask
                    exv = ex_all[:, c_off:c_off + s_c, :]
                    adv = ad_sb[:, t * HD:(t + 1) * HD]
                    nc.vector.tensor_tensor(
                        out=exv, in0=hgv[:, :, DH:DH + HD],
                        in1=adv.unsqueeze(1).broadcast_to([P, s_c, HD]),
                        op=mybir.AluOpType.add)
                    if cfg.lrelu_act:
                        nc.scalar.activation(
                            exv, exv, mybir.ActivationFunctionType.Lrelu,
                            alpha=NEG_SLOPE)
                    else:
                        neg = sm_p.tile([P, cfg.ch, HD], EXDT, tag="neg")
                        negv = neg[:, :s_c, :]
                        nc.vector.tensor_scalar_min(out=negv, in0=exv,
                                                    scalar1=0.0)
                        nc.vector.tensor_scalar_max(out=exv, in0=exv,
                                                    scalar1=0.0)
                        nc.vector.scalar_tensor_tensor(
                            out=exv, in0=negv, scalar=NEG_SLOPE, in1=exv,
                            op0=mybir.AluOpType.mult, op1=mybir.AluOpType.add)
                    nc.scalar.activation(exv, exv,
                                         mybir.ActivationFunctionType.Exp)
                    mk = gmsb[:, gm_off + c_off:gm_off + c_off + s_c]
                    nc.vector.tensor_tensor(
                        out=exv, in0=exv,
                        in1=mk.unsqueeze(2).broadcast_to([P, s_c, HD]),
                        op=mybir.AluOpType.mult)
                c_off += s_c
            gm_off += s_t
            exs = ex_all[:, :s_t, :]
            if cfg.b_level >= 2:
                # normalize ex by the segment sum (folds softmax division
                # into the E x heads tensor, keeping the big mul unscaled)
                den = sm_p.tile([P, HD], F32, tag="den")
                nc.vector.tensor_reduce(
                    den[:], exs.transpose([0, 2, 1]),
                    axis=mybir.AxisListType.X, op=mybir.AluOpType.add)
                rec = sm_p.tile([P, HD], F32, tag="rec")
                nc.vector.reciprocal(rec[:], den[:])
                nc.vector.tensor_tensor(
                    out=exs, in0=exs,
                    in1=rec[:].unsqueeze(1).broadcast_to([P, s_t, HD]),
                    op=mybir.AluOpType.mult)
            one_chunk = len(chunks) == 1
            for hgv, s_c, c_off in hgs:
                if cfg.b_level >= 3:
                    exv = ex_all[:, c_off:c_off + s_c, :]
                    if cfg.ilv:
                        # interleaved: heads innermost, unit stride on all
                        # streams -> DVE 2x mode
                        hgm = hgv[:, :, :DH].rearrange("p s (c h) -> p s c h",
                                                       h=HD)
                        nc.vector.tensor_tensor(
                            out=hgm, in0=hgm,
                            in1=exv.unsqueeze(2)
                                .broadcast_to([P, s_c, cfg.hid, HD]),
                            op=mybir.AluOpType.mult)
                    else:
                        hgm = hgv[:, :, :DH].rearrange("p s (h c) -> p s h c",
                                                       h=HD)
                        nc.vector.tensor_tensor(
                            out=hgm, in0=hgm,
                            in1=exv.unsqueeze(3)
                                .broadcast_to([P, s_c, HD, cfg.hid]),
                            op=mybir.AluOpType.mult)
                if cfg.b_level >= 4:
                    n = s_c
                    while n > 1:
                        k = n // 2
                        nc.vector.tensor_add(
                            hgv[:, 0:k, :DH], hgv[:, 0:k, :DH],
                            hgv[:, n - k:n, :DH])
                        n -= k
                if one_chunk:
                    continue
                if c_off == 0:
                    nc.vector.tensor_copy(agg[:], hgv[:, 0, :DH])
                else:
                    nc.vector.tensor_add(agg[:], agg[:], hgv[:, 0, :DH])
            # y = elu(agg + b1); exp/relu legs on ACT
            aggv = hgs[0][0][:, 0, :DH] if one_chunk else agg[:]
            y = big_p.tile([P, DH], F32, tag="y")
            nc.vector.tensor_add(y[:], aggv, b1sb[:])
            tneg = big_p.tile([P, DH], F32, tag="tneg")
            nc.scalar.activation(tneg[:], y[:],
                                 mybir.ActivationFunctionType.Relu, scale=-1.0)
            nc.scalar.activation(y[:], y[:],
                                 mybir.ActivationFunctionType.Relu)
            nc.scalar.activation(tneg[:], tneg[:],
                                 mybir.ActivationFunctionType.Exp, scale=-1.0)
            nc.vector.scalar_tensor_tensor(
                out=y[:], in0=tneg[:], scalar=-1.0, in1=y[:],
                op0=mybir.AluOpType.add, op1=mybir.AluOpType.add)
            # ---- Phase C: transpose y, layer-2 matmul ----
            if not doC:
                continue
            yT = big_p.tile([P, K2, P], MMDT, tag="yT")
            for k in range(K2):
                pt = psH_p.tile([P, P], F32, tag="psT")
                nc.tensor.transpose(pt[:], y[:, k * P:(k + 1) * P], ident[:])
                nc.scalar.copy(yT[:, k, :], pt[:])
            p2 = psA_p.tile([P, cfg.d_out + 2], F32, tag="ps2")
            for k in range(K2):
                nc.tensor.matmul(p2[:], yT[:, k, :], w2sb[:, k, :],
                                 start=(k == 0), stop=(k == K2 - 1))
            h2sb = out_p.tile([P, cfg.rw2], F32, tag="h2sb")
            nc.scalar.copy(h2sb[:, :cfg.d_out + 1], p2[:, :cfg.d_out + 1])
            nc.vector.memset(h2sb[:, cfg.d_out + 1:], 0.0)
            nc.scalar.copy(ad2_sb[:, t:t + 1], p2[:, cfg.d_out + 1:cfg.d_out + 2])
            nc.sync.dma_start(out=h2own[t * P:(t + 1) * P, :], in_=h2sb[:])

        # ---- AllGather layer-2 table ----
        if doC:
            if cfg.collective:
                nc.gpsimd.collective_compute(
                    "AllGather", mybir.AluOpType.bypass,
                    ins=[h2own[:]], outs=[h2all[:]],
                    replica_groups=[list(range(cfg.n_cores))])
            else:
                nc.sync.dma_start(out=h2all[0:cfg.npc, :], in_=h2own[:])

        # ---- Phase D: layer-2 edge phase ----
        gm_off = 0
        DO = cfg.d_out
        if doB and doC and doD:
            # group tiles into gathers of <=96 slots (12288 idxs): the Q7
            # dma_gather ucode scratch caps num_idxs around 16k
            d_groups = []
            cur, cur_s = [], 0
            for t in range(cfg.tpc):
                if cur and cur_s + cfg.s_prog[t] > 96:
                    d_groups.append(cur)
                    cur, cur_s = [], 0
                cur.append(t)
                cur_s += cfg.s_prog[t]
            if cur:
                d_groups.append(cur)
            group_of_t = {}
            hg2a_of_group = {}
            off = 0
            goffs = {}
            for gi_, grp in enumerate(d_groups):
                gs = sum(cfg.s_prog[t] for t in grp)
                hg2a = hg2_p.tile([P, 96, cfg.rw2], F32, tag="hg2")
                nc.gpsimd.dma_gather(
                    out_ap=hg2a[:, :gs, :],
                    in_ap=h2all[:, :],
                    idxs_ap=gi2sb[:, off * 8:(off + gs) * 8],
                    num_idxs=P * gs, num_idxs_reg=P * gs,
                    elem_size=cfg.rw2, single_packet=False)
                for t in grp:
                    group_of_t[t] = gi_
                hg2a_of_group[gi_] = (hg2a, off)
                goffs[gi_] = off
                off += gs
        if doB and doC and doD:
            for gi_, grp in enumerate(d_groups):
                hg2a, g_off = hg2a_of_group[gi_]
                gs = sum(cfg.s_prog[t] for t in grp)
                e2g = sm_p.tile([P, 96], F32, tag="e2")
                # per-tile alpha_dst add into the group logit buffer
                for t in grp:
                    s_t = cfg.s_prog[t]
                    r0 = gm_off - g_off
                    nc.vector.tensor_tensor(
                        out=e2g[:, r0:r0 + s_t],
                        in0=hg2a[:, r0:r0 + s_t, DO].squeeze(),
                        in1=ad2_sb[:, t:t + 1].broadcast_to([P, s_t]),
                        op=mybir.AluOpType.add)
                    gm_off += s_t
                # group-level leaky-relu + exp + mask
                e2v = e2g[:, :gs]
                if cfg.lrelu_act:
                    nc.scalar.activation(
                        e2v, e2v, mybir.ActivationFunctionType.Lrelu,
                        alpha=NEG_SLOPE)
                else:
                    neg2 = sm_p.tile([P, 96], F32, tag="neg2")
                    nc.vector.tensor_scalar_min(out=neg2[:, :gs], in0=e2v,
                                                scalar1=0.0)
                    nc.vector.tensor_scalar_max(out=e2v, in0=e2v, scalar1=0.0)
                    nc.vector.scalar_tensor_tensor(
                        out=e2v, in0=neg2[:, :gs], scalar=NEG_SLOPE, in1=e2v,
                        op0=mybir.AluOpType.mult, op1=mybir.AluOpType.add)
                nc.scalar.activation(e2v, e2v, mybir.ActivationFunctionType.Exp)
                nc.vector.tensor_tensor(
                    out=e2v, in0=e2v, in1=gmsb[:, g_off:g_off + gs],
                    op=mybir.AluOpType.mult)
                # per-tile softmax normalization folded into e2
                for t in grp:
                    s_t = cfg.s_prog[t]
                    r0 = sum(cfg.s_prog[u] for u in grp if u < t)
                    den2 = sm_p.tile([P, 1], F32, tag="den2s")
                    nc.vector.tensor_reduce(
                        den2[:], e2g[:, r0:r0 + s_t],
                        axis=mybir.AxisListType.X, op=mybir.AluOpType.add)
                    rec2 = sm_p.tile([P, 1], F32, tag="rec2")
                    nc.vector.reciprocal(rec2[:], den2[:])
                    nc.vector.tensor_scalar_mul(
                        out=e2g[:, r0:r0 + s_t], in0=e2g[:, r0:r0 + s_t],
                        scalar1=rec2[:, :1])
                # group-level weighted messages
                nc.vector.tensor_tensor(
                    out=hg2a[:, :gs, :DO], in0=hg2a[:, :gs, :DO],
                    in1=e2v.unsqueeze(2).broadcast_to([P, gs, DO]),
                    op=mybir.AluOpType.mult)
                # per-tile segment sum + bias + store
                for t in grp:
                    s_t = cfg.s_prog[t]
                    r0 = sum(cfg.s_prog[u] for u in grp if u < t)
                    hg2 = hg2a[:, r0:r0 + s_t, :]
                    n = s_t
                    while n > 1:
                        k = n // 2
                        nc.vector.tensor_add(
                            hg2[:, 0:k, :DO], hg2[:, 0:k, :DO],
                            hg2[:, n - k:n, :DO])
                        n -= k
                    osb = out_p.tile([P, DO], F32, tag="osb")
                    nc.vector.tensor_add(osb[:], hg2[:, 0, :DO], b2sb[:])
                    nc.sync.dma_start(out=out[t * P:(t + 1) * P, :], in_=osb[:])

    nc.compile()
    return nc


def default_cfg() -> Cfg:
    return Cfg(n_nodes=10000, n_cores=8, tpc=10, d_in=256, hid=64, heads=8,
               d_out=40, ch=20, mm_mode="f32r", tab_dt="bf16")


def run(inputs: dict, cfg: Cfg | None = None, **run_kwargs):
    cfg = cfg or default_cfg()
    in_maps, node_of_slot = preprocess(cfg, **inputs)
    nc = build_program(cfg)
    res = run_bass_kernel_spmd(nc, in_maps, list(range(cfg.n_cores)),
                               **run_kwargs)
    outs = np.concatenate([res.results[c]["out"] for c in range(cfg.n_cores)],
                          axis=0)
    full = np.zeros((cfg.n_nodes, cfg.d_out), np.float32)
    flat = node_of_slot.reshape(-1)
    real = flat >= 0
    full[flat[real]] = outs[real]
    return full, res


def kernel(**inputs) -> np.ndarray:
    out, _ = run(inputs)
    return out

